# revision 6
# baseline (speedup 1.0000x reference)
"""MoE grouped-linear kernel for Trainium2 (8 NeuronCores, expert-parallel).

y[t] = weight[expert_ids[t]] @ x[t] + bias[expert_ids[t]]
T=131072 tokens, E=64 experts, I=O=512, global per-expert capacity 3072
(overflow -> 0, matching the reference's capacity-bucketed dispatch).

Sharding: expert-parallel, count-adaptive. The host computes the routing
(argsort by expert), sorts experts by token count and assigns rank r to
(slot r//8, core r%8) so the 8 experts sharing a slot have similar counts;
slot k is compiled with nt[k] = ceil(max_count/128) token-tiles (the
program is built per nt-tuple and cached). Each expert's tokens are
gathered and pre-transposed on the host into the SBUF matmul layout
[128 i_lo, tile, 4 i_chunk, 128 tok_lo] fp16, so the device runs pure
dense GEMMs with no on-chip gather/scatter/transpose:

  per slot k (nt[k] token-tiles of 128):
    - one contiguous HWDGE load of X^T (SP ring; prefetched SKEW ahead;
      the first slot's load is split so matmuls start after ~0.5 MB),
    - per tile: 4 fp16 matmuls (X^T chunk stationary, W^T streaming,
      N=512) accumulate into one fp32 PSUM bank -- back-to-back warm
      matmuls at the 216 ns streaming roofline,
    - DVE evicts PSUM -> fp16 SBUF, fusing the fp32 bias add,
    - the result block is stored in ~6-tile chunks (ACT ring, separate
      from the SP load ring) so the final store tail is short.
  Weights/bias load on the ACT ring during the prologue, interleaved so
  slot 0's arrive first.

The host scatters the fp16 result blocks back to token order and upcasts
to fp32. Tokens past a slot's device capacity (pos in [2304, 3072)) are
computed exactly on the host (~never happens for uniform routing); tokens
past the global capacity 3072 are 0 like the reference.
"""

import os
import sys

sys.path.insert(0, "/opt/trn_rl_repo")

import numpy as np

T, D, E, NC = 131072, 512, 64, 8
EL = E // NC      # experts per core (= number of slots)
CAPD = 2304       # max device per-expert capacity (18 tiles of 128)
NTMAX = CAPD // 128
CAPG = 3072       # reference global per-expert capacity
SKEW = 3          # x prefetch depth (slots)
P = 128

_cache = {}
last_result = None


def _build_program(nt_slot):
    from concourse import bacc, mybir, tile

    f32 = mybir.dt.float32
    f16 = mybir.dt.float16
    ntot = sum(nt_slot)
    off = [0]
    for nt in nt_slot:
        off.append(off[-1] + nt)

    nc = bacc.Bacc(
        "TRN2",
        target_bir_lowering=False,
        debug=False,
        enable_asserts=False,
        num_devices=NC,
    )
    x_d = nc.dram_tensor("x", [P, ntot * 512], f16, kind="ExternalInput").ap()
    w_d = nc.dram_tensor("wt", [EL, P, 4 * D], f16, kind="ExternalInput").ap()
    b_d = nc.dram_tensor("bias", [EL, P, D], f32, kind="ExternalInput").ap()
    y_d = nc.dram_tensor("y", [P, ntot * 512], f16, kind="ExternalOutput").ap()

    with tile.TileContext(nc) as tc:
        with (
            tc.tile_pool(name="wt", bufs=1) as wtp,
            tc.tile_pool(name="bt", bufs=1) as btp,
            tc.tile_pool(name="xg0", bufs=1) as xg0p,
            tc.tile_pool(name="xg", bufs=SKEW + 1) as xgp,
            tc.tile_pool(name="ys", bufs=8) as ysp,
            tc.tile_pool(name="psY", bufs=8, space="PSUM") as psYp,
        ):
            def load_x(k):
                nt = nt_slot[k]
                if k == 0:
                    # split so the first matmuls wait on ~0.5 MB, not 2.25 MB
                    n0 = min(4, nt)
                    ta = xg0p.tile([P, n0 * 512], f16, tag="xga")
                    nc.sync.dma_start(out=ta[:], in_=x_d[:, : n0 * 512])
                    segs = [(ta, 0, n0)]
                    if nt > n0:
                        tb = xg0p.tile([P, (nt - n0) * 512], f16, tag="xgb")
                        nc.sync.dma_start(
                            out=tb[:], in_=x_d[:, n0 * 512 : nt * 512]
                        )
                        segs.append((tb, n0, nt - n0))
                    return segs
                t = xgp.tile([P, NTMAX * 512], f16, tag="xg")
                nc.sync.dma_start(
                    out=t[:, : nt * 512],
                    in_=x_d[:, off[k] * 512 : (off[k] + nt) * 512],
                )
                return [(t, 0, nt)]

            # prologue: x on the SP ring. ACT ring: slot-0 weights/bias
            # first (compute starts after ~1.25 MB), then the remaining
            # slots' weights/bias as two large DMAs, then the y stores.
            pend = [load_x(0)]
            w0 = wtp.tile([P, 4 * D], f16, tag="w0")
            nc.scalar.dma_start(out=w0[:], in_=w_d[0])
            b0 = btp.tile([P, D], f32, tag="b0")
            nc.scalar.dma_start(out=b0[:], in_=b_d[0])
            wR = wtp.tile([P, (EL - 1) * 4 * D], f16, tag="wR")
            nc.scalar.dma_start(
                out=wR[:].rearrange("p (e d) -> p e d", d=4 * D),
                in_=w_d[1:].rearrange("e p d -> p e d"),
            )
            bR = btp.tile([P, (EL - 1) * D], f32, tag="bR")
            nc.scalar.dma_start(
                out=bR[:].rearrange("p (e d) -> p e d", d=D),
                in_=b_d[1:].rearrange("e p d -> p e d"),
            )
            for k in range(1, SKEW):
                pend.append(load_x(k))

            for k in range(EL):
                segs = pend.pop(0)
                nt = nt_slot[k]
                w_k = w0 if k == 0 else wR[:, (k - 1) * 4 * D : k * 4 * D]
                b_k = b0 if k == 0 else bR[:, (k - 1) * D : k * D]
                chunk = 4 if k == EL - 1 else 6  # store chunk (tiles)
                ys = None
                done = 0
                for xt_t, bt0, nbt in segs:
                    for bi in range(nbt):
                        bt = bt0 + bi
                        if ys is None:
                            ys = ysp.tile([P, chunk * D], f16, tag="ys")
                        psY = psYp.tile([P, D], f32, tag="psY")
                        for j in range(4):
                            nc.tensor.matmul(
                                out=psY[:],
                                lhsT=xt_t[:, bi * 512 + j * P : bi * 512 + (j + 1) * P],
                                rhs=w_k[:, j * D : (j + 1) * D],
                                start=(j == 0),
                                stop=(j == 3),
                            )
                        nc.vector.tensor_add(
                            out=ys[:, (bt - done) * D : (bt - done + 1) * D],
                            in0=psY[:],
                            in1=b_k[:],
                        )
                        if bt + 1 == nt or (bt + 1) % chunk == 0:
                            nc.scalar.dma_start(
                                out=y_d[:, (off[k] + done) * 512 : (off[k] + bt + 1) * 512],
                                in_=ys[:, : (bt + 1 - done) * D],
                            )
                            done = bt + 1
                            ys = None
                if k + SKEW < EL:
                    pend.append(load_x(k + SKEW))
    nc.compile()
    return nc


def _ensure_ntff_hook():
    """The agent image's antenv lacks axon_hooks; shim it and install the
    ctypes NTFF profiling hook so trace=True works under axon."""
    import types

    try:
        from antenv import axon_hooks  # noqa: F401
        return
    except ImportError:
        pass
    mod = types.ModuleType("antenv.axon_hooks")
    _h = {"hook": None}
    mod.set_axon_ntff_profile_hook = lambda h: _h.update(hook=h)
    mod.get_axon_ntff_profile_hook = lambda: _h["hook"]
    sys.modules["antenv.axon_hooks"] = mod
    import antenv

    antenv.axon_hooks = mod
    try:
        if "/root/.axon_site" not in sys.path:
            sys.path.insert(0, "/root/.axon_site")
        from trn_agent_boot.trn_boot import _ntff_profile_via_ctypes

        hook = _ntff_profile_via_ctypes("/opt/axon/libaxon_pjrt.so")
        if hook is not None:
            mod.set_axon_ntff_profile_hook(hook)
    except Exception:
        pass


def kernel(x, weight, bias, expert_ids):
    global last_result
    from concourse import bass_utils
    from concourse.bass_utils import run_bass_kernel_spmd

    x = np.asarray(x, dtype=np.float32)
    weight = np.asarray(weight, dtype=np.float32)
    bias = np.asarray(bias, dtype=np.float32)
    expert_ids = np.asarray(expert_ids, dtype=np.int32)

    # ---- host routing: tokens sorted by expert, position within expert ----
    order = np.argsort(expert_ids, kind="stable")
    ids_s = expert_ids[order]
    counts = np.bincount(expert_ids, minlength=E)
    starts = np.cumsum(counts) - counts
    pos_s = np.arange(T, dtype=np.int64) - starts[ids_s]
    sel = pos_s < CAPD  # tokens the device computes

    # sort experts by count desc; rank r -> (slot r//NC, core r%NC)
    counts_c = np.minimum(counts, CAPD)
    rank = np.argsort(-counts_c, kind="stable")
    perm = rank.reshape(EL, NC)  # perm[slot, core] = expert id
    nt_slot = tuple(
        max(1, int(-(-counts_c[perm[k]].max() // 128))) for k in range(EL)
    )
    off = [0]
    for nt in nt_slot:
        off.append(off[-1] + nt)
    ntot = off[-1]

    if nt_slot not in _cache:
        _cache[nt_slot] = _build_program(nt_slot)
    nc = _cache[nt_slot]

    # ---- pack x: [E, CAPD, D] fp16, then to [E, 128 i_lo, bt, j, 128 t_lo] ----
    x16 = x.astype(np.float16)
    buf = np.zeros((E, CAPD, D), np.float16)
    buf[ids_s[sel], pos_s[sel]] = x16[order[sel]]
    xt = np.ascontiguousarray(
        buf.reshape(E, NTMAX, P, 4, P).transpose(0, 4, 1, 3, 2)
    ).reshape(E, P, NTMAX * 512)

    # ---- weights: [E, O, I] -> W^T tile layout [E, 128 i_lo, 4 j * 512 o] ----
    wt16 = np.ascontiguousarray(weight.transpose(0, 2, 1)).astype(np.float16)
    wt16 = np.ascontiguousarray(
        wt16.reshape(E, 4, P, D).transpose(0, 2, 1, 3)
    ).reshape(E, P, 4 * D)

    in_maps = []
    for c in range(NC):
        ex = perm[:, c]
        in_maps.append(
            {
                "x": np.concatenate(
                    [xt[ex[k]][:, : nt_slot[k] * 512] for k in range(EL)], axis=1
                ),
                "wt": np.ascontiguousarray(wt16[ex]),
                "bias": np.ascontiguousarray(
                    np.broadcast_to(bias[ex][:, None, :], (EL, P, D))
                ),
            }
        )

    trace = bool(int(os.environ.get("KERNEL_TRACE", "0")))
    kwargs = {}
    if trace:
        _ensure_ntff_hook()
        bass_utils.upload_artifacts = lambda tmpdir: "local://" + tmpdir
        tdir = os.environ.get("KERNEL_TRACE_DIR")
        if tdir:
            os.makedirs(tdir, exist_ok=True)
            kwargs["tmpdir"] = tdir
    res = run_bass_kernel_spmd(
        nc, in_maps, core_ids=list(range(NC)), trace=trace, **kwargs
    )
    last_result = res

    # ---- unpack: y blocks [128 t_lo, bt*512+o] per (slot, core) -> [E, CAPD, D]
    ypad = np.zeros((E, P, NTMAX * 512), np.float16)
    for c in range(NC):
        yc = res.results[c]["y"]
        for k in range(EL):
            ypad[perm[k, c]][:, : nt_slot[k] * 512] = yc[
                :, off[k] * 512 : (off[k] + nt_slot[k]) * 512
            ]
    yall = (
        ypad.reshape(E, P, NTMAX, D).transpose(0, 2, 1, 3).reshape(E, CAPD, D)
    )
    out = np.zeros((T, D), np.float32)
    out[order[sel]] = yall[ids_s[sel], pos_s[sel]].astype(np.float32)

    # tokens beyond device capacity but within global capacity: exact host math
    ovf = (~sel) & (pos_s < CAPG)
    for t_idx in order[ovf]:
        e = expert_ids[t_idx]
        out[t_idx] = weight[e] @ x[t_idx] + bias[e]
    return out


# revision 9
# speedup vs baseline: 1.0136x; 1.0136x over previous
"""MoE grouped-linear kernel for Trainium2 (8 NeuronCores, expert-parallel).

y[t] = weight[expert_ids[t]] @ x[t] + bias[expert_ids[t]]
T=131072 tokens, E=64 experts, I=O=512, global per-expert capacity 3072
(overflow -> 0, matching the reference's capacity-bucketed dispatch).

Sharding: expert-parallel, count-adaptive. The host computes the routing
(argsort by expert), sorts experts by token count and assigns rank r to
(slot r//8, core r%8) so the 8 experts sharing a slot have similar counts;
slot k is compiled with nt[k] = ceil(max_count/128) token-tiles (the
program is built per nt-tuple and cached). Each expert's tokens are
gathered and pre-transposed on the host into the SBUF matmul layout
[128 i_lo, tile, 4 i_chunk, 128 tok_lo] fp16, so the device runs pure
dense GEMMs with no on-chip gather/scatter/transpose:

  per slot k (nt[k] token-tiles of 128):
    - one contiguous HWDGE load of X^T (SP ring; prefetched SKEW ahead;
      the first slot's load is split so matmuls start after ~0.5 MB),
    - per tile: 4 fp16 matmuls (X^T chunk stationary, W^T streaming,
      N=512) accumulate into one fp32 PSUM bank -- back-to-back warm
      matmuls at the 216 ns streaming roofline,
    - DVE evicts PSUM -> fp16 SBUF, fusing the fp32 bias add,
    - the result block is stored in ~6-tile chunks (ACT ring, separate
      from the SP load ring) so the final store tail is short.
  Weights/bias load on the ACT ring during the prologue, interleaved so
  slot 0's arrive first.

The host scatters the fp16 result blocks back to token order and upcasts
to fp32. Tokens past a slot's device capacity (pos in [2304, 3072)) are
computed exactly on the host (~never happens for uniform routing); tokens
past the global capacity 3072 are 0 like the reference.
"""

import os
import sys

sys.path.insert(0, "/opt/trn_rl_repo")

import numpy as np

T, D, E, NC = 131072, 512, 64, 8
EL = E // NC      # experts per core (= number of slots)
CAPD = 2304       # max device per-expert capacity (18 tiles of 128)
NTMAX = CAPD // 128
CAPG = 3072       # reference global per-expert capacity
SKEW = 3          # x prefetch depth (slots)
P = 128

_cache = {}
last_result = None


def _build_program(nt_slot):
    from concourse import bacc, mybir, tile

    f32 = mybir.dt.float32
    f16 = mybir.dt.float16
    ntot = sum(nt_slot)
    off = [0]
    for nt in nt_slot:
        off.append(off[-1] + nt)

    nc = bacc.Bacc(
        "TRN2",
        target_bir_lowering=False,
        debug=False,
        enable_asserts=False,
        num_devices=NC,
    )
    x_d = nc.dram_tensor("x", [P, ntot * 512], f16, kind="ExternalInput").ap()
    w_d = nc.dram_tensor("wt", [P, EL * 4 * D], f16, kind="ExternalInput").ap()
    b_d = nc.dram_tensor("bias", [P, EL * D], f32, kind="ExternalInput").ap()
    y_d = nc.dram_tensor("y", [P, ntot * 512], f16, kind="ExternalOutput").ap()

    with tile.TileContext(nc) as tc:
        with (
            tc.tile_pool(name="wt", bufs=1) as wtp,
            tc.tile_pool(name="bt", bufs=1) as btp,
            tc.tile_pool(name="xg0", bufs=1) as xg0p,
            tc.tile_pool(name="xg", bufs=SKEW + 1) as xgp,
            tc.tile_pool(name="ys", bufs=8) as ysp,
            tc.tile_pool(name="psY", bufs=8, space="PSUM") as psYp,
        ):
            def load_x(k):
                nt = nt_slot[k]
                if k == 0:
                    # split so the first matmuls wait on ~0.5 MB, not 2.25 MB
                    n0 = min(4, nt)
                    ta = xg0p.tile([P, n0 * 512], f16, tag="xga")
                    nc.sync.dma_start(out=ta[:], in_=x_d[:, : n0 * 512])
                    segs = [(ta, 0, n0)]
                    if nt > n0:
                        tb = xg0p.tile([P, (nt - n0) * 512], f16, tag="xgb")
                        nc.sync.dma_start(
                            out=tb[:], in_=x_d[:, n0 * 512 : nt * 512]
                        )
                        segs.append((tb, n0, nt - n0))
                    return segs
                t = xgp.tile([P, NTMAX * 512], f16, tag="xg")
                nc.sync.dma_start(
                    out=t[:, : nt * 512],
                    in_=x_d[:, off[k] * 512 : (off[k] + nt) * 512],
                )
                return [(t, 0, nt)]

            # prologue: x on the SP ring. ACT ring: slot-0 weights/bias
            # first (compute starts after ~1.25 MB), then the remaining
            # slots' weights/bias as two large DMAs, then the y stores.
            pend = [load_x(0)]
            w0 = wtp.tile([P, 4 * D], f16, tag="w0")
            nc.scalar.dma_start(out=w0[:], in_=w_d[:, : 4 * D])
            b0 = btp.tile([P, D], f32, tag="b0")
            nc.scalar.dma_start(out=b0[:], in_=b_d[:, :D])
            wR = wtp.tile([P, (EL - 1) * 4 * D], f16, tag="wR")
            nc.scalar.dma_start(out=wR[:], in_=w_d[:, 4 * D :])
            bR = btp.tile([P, (EL - 1) * D], f32, tag="bR")
            nc.scalar.dma_start(out=bR[:], in_=b_d[:, D:])
            for k in range(1, SKEW):
                pend.append(load_x(k))

            for k in range(EL):
                segs = pend.pop(0)
                nt = nt_slot[k]
                w_k = w0 if k == 0 else wR[:, (k - 1) * 4 * D : k * 4 * D]
                b_k = b0 if k == 0 else bR[:, (k - 1) * D : k * D]
                chunk = 4 if k == EL - 1 else 6  # store chunk (tiles)
                ys = None
                done = 0
                for xt_t, bt0, nbt in segs:
                    for bi in range(nbt):
                        bt = bt0 + bi
                        if ys is None:
                            ys = ysp.tile([P, chunk * D], f16, tag="ys")
                        psY = psYp.tile([P, D], f32, tag="psY")
                        for j in range(4):
                            nc.tensor.matmul(
                                out=psY[:],
                                lhsT=xt_t[:, bi * 512 + j * P : bi * 512 + (j + 1) * P],
                                rhs=w_k[:, j * D : (j + 1) * D],
                                start=(j == 0),
                                stop=(j == 3),
                            )
                        nc.vector.tensor_add(
                            out=ys[:, (bt - done) * D : (bt - done + 1) * D],
                            in0=psY[:],
                            in1=b_k[:],
                        )
                        if bt + 1 == nt or (bt + 1) % chunk == 0:
                            nc.scalar.dma_start(
                                out=y_d[:, (off[k] + done) * 512 : (off[k] + bt + 1) * 512],
                                in_=ys[:, : (bt + 1 - done) * D],
                            )
                            done = bt + 1
                            ys = None
                if k + SKEW < EL:
                    pend.append(load_x(k + SKEW))
    nc.compile()
    return nc


def _ensure_ntff_hook():
    """The agent image's antenv lacks axon_hooks; shim it and install the
    ctypes NTFF profiling hook so trace=True works under axon."""
    import types

    try:
        from antenv import axon_hooks  # noqa: F401
        return
    except ImportError:
        pass
    mod = types.ModuleType("antenv.axon_hooks")
    _h = {"hook": None}
    mod.set_axon_ntff_profile_hook = lambda h: _h.update(hook=h)
    mod.get_axon_ntff_profile_hook = lambda: _h["hook"]
    sys.modules["antenv.axon_hooks"] = mod
    import antenv

    antenv.axon_hooks = mod
    try:
        if "/root/.axon_site" not in sys.path:
            sys.path.insert(0, "/root/.axon_site")
        from trn_agent_boot.trn_boot import _ntff_profile_via_ctypes

        hook = _ntff_profile_via_ctypes("/opt/axon/libaxon_pjrt.so")
        if hook is not None:
            mod.set_axon_ntff_profile_hook(hook)
    except Exception:
        pass


def kernel(x, weight, bias, expert_ids):
    global last_result
    from concourse import bass_utils
    from concourse.bass_utils import run_bass_kernel_spmd

    x = np.asarray(x, dtype=np.float32)
    weight = np.asarray(weight, dtype=np.float32)
    bias = np.asarray(bias, dtype=np.float32)
    expert_ids = np.asarray(expert_ids, dtype=np.int32)

    # ---- host routing: tokens sorted by expert, position within expert ----
    order = np.argsort(expert_ids, kind="stable")
    ids_s = expert_ids[order]
    counts = np.bincount(expert_ids, minlength=E)
    starts = np.cumsum(counts) - counts
    pos_s = np.arange(T, dtype=np.int64) - starts[ids_s]
    sel = pos_s < CAPD  # tokens the device computes

    # sort experts by count desc; rank r -> (slot r//NC, core r%NC)
    counts_c = np.minimum(counts, CAPD)
    rank = np.argsort(-counts_c, kind="stable")
    perm = rank.reshape(EL, NC)  # perm[slot, core] = expert id
    nt_slot = tuple(
        max(1, int(-(-counts_c[perm[k]].max() // 128))) for k in range(EL)
    )
    off = [0]
    for nt in nt_slot:
        off.append(off[-1] + nt)
    ntot = off[-1]

    if nt_slot not in _cache:
        _cache[nt_slot] = _build_program(nt_slot)
    nc = _cache[nt_slot]

    # ---- pack x: [E, CAPD, D] fp16, then to [E, 128 i_lo, bt, j, 128 t_lo] ----
    x16 = x.astype(np.float16)
    buf = np.zeros((E, CAPD, D), np.float16)
    buf[ids_s[sel], pos_s[sel]] = x16[order[sel]]
    xt = np.ascontiguousarray(
        buf.reshape(E, NTMAX, P, 4, P).transpose(0, 4, 1, 3, 2)
    ).reshape(E, P, NTMAX * 512)

    # ---- weights: [E, O, I] -> W^T tile layout [E, 128 i_lo, 4 j * 512 o] ----
    wt16 = np.ascontiguousarray(weight.transpose(0, 2, 1)).astype(np.float16)
    wt16 = np.ascontiguousarray(
        wt16.reshape(E, 4, P, D).transpose(0, 2, 1, 3)
    ).reshape(E, P, 4 * D)

    in_maps = []
    for c in range(NC):
        ex = perm[:, c]
        in_maps.append(
            {
                "x": np.concatenate(
                    [xt[ex[k]][:, : nt_slot[k] * 512] for k in range(EL)], axis=1
                ),
                "wt": np.ascontiguousarray(
                    wt16[ex].transpose(1, 0, 2).reshape(P, EL * 4 * D)
                ),
                "bias": np.ascontiguousarray(
                    np.broadcast_to(bias[ex].reshape(1, EL * D), (P, EL * D))
                ),
            }
        )

    trace = bool(int(os.environ.get("KERNEL_TRACE", "0")))
    kwargs = {}
    if trace:
        _ensure_ntff_hook()
        bass_utils.upload_artifacts = lambda tmpdir: "local://" + tmpdir
        tdir = os.environ.get("KERNEL_TRACE_DIR")
        if tdir:
            os.makedirs(tdir, exist_ok=True)
            kwargs["tmpdir"] = tdir
    res = run_bass_kernel_spmd(
        nc, in_maps, core_ids=list(range(NC)), trace=trace, **kwargs
    )
    last_result = res

    # ---- unpack: y blocks [128 t_lo, bt*512+o] per (slot, core) -> [E, CAPD, D]
    ypad = np.zeros((E, P, NTMAX * 512), np.float16)
    for c in range(NC):
        yc = res.results[c]["y"]
        for k in range(EL):
            ypad[perm[k, c]][:, : nt_slot[k] * 512] = yc[
                :, off[k] * 512 : (off[k] + nt_slot[k]) * 512
            ]
    yall = (
        ypad.reshape(E, P, NTMAX, D).transpose(0, 2, 1, 3).reshape(E, CAPD, D)
    )
    out = np.zeros((T, D), np.float32)
    out[order[sel]] = yall[ids_s[sel], pos_s[sel]].astype(np.float32)

    # tokens beyond device capacity but within global capacity: exact host math
    ovf = (~sel) & (pos_s < CAPG)
    for t_idx in order[ovf]:
        e = expert_ids[t_idx]
        out[t_idx] = weight[e] @ x[t_idx] + bias[e]
    return out


# revision 14
# speedup vs baseline: 1.1164x; 1.1014x over previous
"""MoE grouped-linear kernel for Trainium2 (8 NeuronCores, expert-parallel).

y[t] = weight[expert_ids[t]] @ x[t] + bias[expert_ids[t]]
T=131072 tokens, E=64 experts, I=O=512, global per-expert capacity 3072
(overflow -> 0, matching the reference's capacity-bucketed dispatch).

Sharding: expert-parallel, count-adaptive. The host computes the routing
(argsort by expert), sorts experts by token count and assigns rank r to
(slot r//8, core r%8) so the 8 experts sharing a slot have similar counts;
slot k is compiled with nt[k] = ceil(max_count/128) token-tiles (the
program is built per nt-tuple and cached). Each expert's tokens are
gathered and pre-transposed on the host into the SBUF matmul layout
[128 i_lo, tile, 4 i_chunk, 128 tok_lo] fp16, so the device runs pure
dense GEMMs with no on-chip gather/scatter/transpose:

  per slot k (nt[k] token-tiles of 128):
    - one contiguous HWDGE load of X^T (SP ring; prefetched SKEW ahead;
      the first slot's load is split so matmuls start after ~0.5 MB),
    - per tile: 4 fp16 matmuls (X^T chunk stationary, W^T streaming,
      N=512) accumulate into one fp32 PSUM bank -- back-to-back warm
      matmuls at the 216 ns streaming roofline,
    - DVE evicts PSUM -> fp16 SBUF, fusing the fp32 bias add,
    - the result block is stored in ~6-tile chunks (ACT ring, separate
      from the SP load ring) so the final store tail is short.
  Weights/bias load on the ACT ring during the prologue, interleaved so
  slot 0's arrive first.

The host scatters the fp16 result blocks back to token order and upcasts
to fp32. Tokens past a slot's device capacity (pos in [2304, 3072)) are
computed exactly on the host (~never happens for uniform routing); tokens
past the global capacity 3072 are 0 like the reference.
"""

import os
import sys

sys.path.insert(0, "/opt/trn_rl_repo")

import numpy as np

T, D, E, NC = 131072, 512, 64, 8
EL = E // NC      # experts per core (= number of slots)
CAPD = 2304       # max device per-expert capacity (18 tiles of 128)
NTMAX = CAPD // 128
CAPG = 3072       # reference global per-expert capacity
SKEW = 3          # x prefetch depth (slots)
P = 128

_cache = {}
last_result = None


def _build_program(nt_slot):
    from concourse import bacc, mybir, tile

    f32 = mybir.dt.float32
    f16 = mybir.dt.float16
    ntot = sum(nt_slot)
    off = [0]
    for nt in nt_slot:
        off.append(off[-1] + nt)

    nc = bacc.Bacc(
        "TRN2",
        target_bir_lowering=False,
        debug=False,
        enable_asserts=False,
        num_devices=NC,
    )
    x_d = nc.dram_tensor("x", [P, ntot * 512], f16, kind="ExternalInput").ap()
    w_d = nc.dram_tensor("wt", [P, EL * 4 * D], f16, kind="ExternalInput").ap()
    b_d = nc.dram_tensor("bias", [1, EL * D], f16, kind="ExternalInput").ap()
    y_d = nc.dram_tensor("y", [P, ntot * 512], f16, kind="ExternalOutput").ap()

    with tile.TileContext(nc) as tc:
        with (
            tc.tile_pool(name="const", bufs=1) as constp,
            tc.tile_pool(name="wt", bufs=3) as wtp,
            tc.tile_pool(name="bt", bufs=2) as btp,
            tc.tile_pool(name="xg0", bufs=1) as xg0p,
            tc.tile_pool(name="xg", bufs=SKEW + 1) as xgp,
            tc.tile_pool(name="ys", bufs=8) as ysp,
            tc.tile_pool(name="psY", bufs=8, space="PSUM") as psYp,
        ):
            def load_x(k):
                nt = nt_slot[k]
                if k == 0:
                    # split so the first matmuls wait on ~0.5 MB, not 2.25 MB
                    n0 = min(4, nt)
                    ta = xg0p.tile([P, n0 * 512], f16, tag="xga")
                    nc.sync.dma_start(out=ta[:], in_=x_d[:, : n0 * 512])
                    segs = [(ta, 0, n0)]
                    if nt > n0:
                        tb = xg0p.tile([P, (nt - n0) * 512], f16, tag="xgb")
                        nc.sync.dma_start(
                            out=tb[:], in_=x_d[:, n0 * 512 : nt * 512]
                        )
                        segs.append((tb, n0, nt - n0))
                    return segs
                t = xgp.tile([P, NTMAX * 512], f16, tag="xg")
                nc.sync.dma_start(
                    out=t[:, : nt * 512],
                    in_=x_d[:, off[k] * 512 : (off[k] + nt) * 512],
                )
                return [(t, 0, nt)]

            # prologue: x on the SP ring. ACT ring: raw bias (16 KB), then
            # per-slot weight loads just-in-time (WSKEW slots ahead),
            # interleaved with the y stores. Bias is replicated across the
            # 128 partitions on-chip via a K=1 matmul per slot.
            ones_t = constp.tile([1, P], f16)
            nc.gpsimd.memset(ones_t[:], 1.0)
            braw = constp.tile([1, EL * D], f16)
            nc.scalar.dma_start(out=braw[:], in_=b_d)

            WSKEW = 3

            def load_w(k):
                w = wtp.tile([P, 4 * D], f16, tag="wt")
                nc.scalar.dma_start(
                    out=w[:], in_=w_d[:, k * 4 * D : (k + 1) * 4 * D]
                )
                return w

            pend = [load_x(0)]
            wpend = [load_w(0)]
            for k in range(1, SKEW):
                pend.append(load_x(k))
            for k in range(1, WSKEW):
                wpend.append(load_w(k))

            for k in range(EL):
                segs = pend.pop(0)
                w_k = wpend.pop(0)
                nt = nt_slot[k]
                # replicate bias[k] across partitions: psum = ones^T @ braw_k
                psB = psYp.tile([P, D], f32, tag="psY")
                nc.tensor.matmul(
                    out=psB[:],
                    lhsT=ones_t[:],
                    rhs=braw[:, k * D : (k + 1) * D],
                    start=True,
                    stop=True,
                )
                b_k = btp.tile([P, D], f32, tag="bt")
                nc.vector.tensor_copy(out=b_k[:], in_=psB[:])
                chunk = 4 if k == EL - 1 else 6  # store chunk (tiles)
                ys = None
                done = 0
                for xt_t, bt0, nbt in segs:
                    for bi in range(nbt):
                        bt = bt0 + bi
                        if ys is None:
                            ys = ysp.tile([P, chunk * D], f16, tag="ys")
                        psY = psYp.tile([P, D], f32, tag="psY")
                        for j in range(4):
                            nc.tensor.matmul(
                                out=psY[:],
                                lhsT=xt_t[:, bi * 512 + j * P : bi * 512 + (j + 1) * P],
                                rhs=w_k[:, j * D : (j + 1) * D],
                                start=(j == 0),
                                stop=(j == 3),
                            )
                        nc.vector.tensor_add(
                            out=ys[:, (bt - done) * D : (bt - done + 1) * D],
                            in0=psY[:],
                            in1=b_k[:],
                        )
                        if bt + 1 == nt or (bt + 1) % chunk == 0:
                            nc.scalar.dma_start(
                                out=y_d[:, (off[k] + done) * 512 : (off[k] + bt + 1) * 512],
                                in_=ys[:, : (bt + 1 - done) * D],
                            )
                            done = bt + 1
                            ys = None
                if k + SKEW < EL:
                    pend.append(load_x(k + SKEW))
                if k + WSKEW < EL:
                    wpend.append(load_w(k + WSKEW))
    nc.compile()
    return nc


def _ensure_ntff_hook():
    """The agent image's antenv lacks axon_hooks; shim it and install the
    ctypes NTFF profiling hook so trace=True works under axon."""
    import types

    try:
        from antenv import axon_hooks  # noqa: F401
        return
    except ImportError:
        pass
    mod = types.ModuleType("antenv.axon_hooks")
    _h = {"hook": None}
    mod.set_axon_ntff_profile_hook = lambda h: _h.update(hook=h)
    mod.get_axon_ntff_profile_hook = lambda: _h["hook"]
    sys.modules["antenv.axon_hooks"] = mod
    import antenv

    antenv.axon_hooks = mod
    try:
        if "/root/.axon_site" not in sys.path:
            sys.path.insert(0, "/root/.axon_site")
        from trn_agent_boot.trn_boot import _ntff_profile_via_ctypes

        hook = _ntff_profile_via_ctypes("/opt/axon/libaxon_pjrt.so")
        if hook is not None:
            mod.set_axon_ntff_profile_hook(hook)
    except Exception:
        pass


def kernel(x, weight, bias, expert_ids):
    global last_result
    from concourse import bass_utils
    from concourse.bass_utils import run_bass_kernel_spmd

    x = np.asarray(x, dtype=np.float32)
    weight = np.asarray(weight, dtype=np.float32)
    bias = np.asarray(bias, dtype=np.float32)
    expert_ids = np.asarray(expert_ids, dtype=np.int32)

    # ---- host routing: tokens sorted by expert, position within expert ----
    order = np.argsort(expert_ids, kind="stable")
    ids_s = expert_ids[order]
    counts = np.bincount(expert_ids, minlength=E)
    starts = np.cumsum(counts) - counts
    pos_s = np.arange(T, dtype=np.int64) - starts[ids_s]
    sel = pos_s < CAPD  # tokens the device computes

    # sort experts by count desc; rank r -> (slot r//NC, core r%NC)
    counts_c = np.minimum(counts, CAPD)
    rank = np.argsort(-counts_c, kind="stable")
    perm = rank.reshape(EL, NC)  # perm[slot, core] = expert id
    nt_slot = tuple(
        max(1, int(-(-counts_c[perm[k]].max() // 128))) for k in range(EL)
    )
    off = [0]
    for nt in nt_slot:
        off.append(off[-1] + nt)
    ntot = off[-1]

    if nt_slot not in _cache:
        _cache[nt_slot] = _build_program(nt_slot)
    nc = _cache[nt_slot]

    # ---- pack x: [E, CAPD, D] fp16, then to [E, 128 i_lo, bt, j, 128 t_lo] ----
    x16 = x.astype(np.float16)
    buf = np.zeros((E, CAPD, D), np.float16)
    buf[ids_s[sel], pos_s[sel]] = x16[order[sel]]
    xt = np.ascontiguousarray(
        buf.reshape(E, NTMAX, P, 4, P).transpose(0, 4, 1, 3, 2)
    ).reshape(E, P, NTMAX * 512)

    # ---- weights: [E, O, I] -> W^T tile layout [E, 128 i_lo, 4 j * 512 o] ----
    wt16 = np.ascontiguousarray(weight.transpose(0, 2, 1)).astype(np.float16)
    wt16 = np.ascontiguousarray(
        wt16.reshape(E, 4, P, D).transpose(0, 2, 1, 3)
    ).reshape(E, P, 4 * D)

    in_maps = []
    for c in range(NC):
        ex = perm[:, c]
        in_maps.append(
            {
                "x": np.concatenate(
                    [xt[ex[k]][:, : nt_slot[k] * 512] for k in range(EL)], axis=1
                ),
                "wt": np.ascontiguousarray(
                    wt16[ex].transpose(1, 0, 2).reshape(P, EL * 4 * D)
                ),
                "bias": np.ascontiguousarray(
                    bias[ex].reshape(1, EL * D).astype(np.float16)
                ),
            }
        )

    trace = bool(int(os.environ.get("KERNEL_TRACE", "0")))
    kwargs = {}
    if trace:
        _ensure_ntff_hook()
        bass_utils.upload_artifacts = lambda tmpdir: "local://" + tmpdir
        tdir = os.environ.get("KERNEL_TRACE_DIR")
        if tdir:
            os.makedirs(tdir, exist_ok=True)
            kwargs["tmpdir"] = tdir
    res = run_bass_kernel_spmd(
        nc, in_maps, core_ids=list(range(NC)), trace=trace, **kwargs
    )
    last_result = res

    # ---- unpack: y blocks [128 t_lo, bt*512+o] per (slot, core) -> [E, CAPD, D]
    ypad = np.zeros((E, P, NTMAX * 512), np.float16)
    for c in range(NC):
        yc = res.results[c]["y"]
        for k in range(EL):
            ypad[perm[k, c]][:, : nt_slot[k] * 512] = yc[
                :, off[k] * 512 : (off[k] + nt_slot[k]) * 512
            ]
    yall = (
        ypad.reshape(E, P, NTMAX, D).transpose(0, 2, 1, 3).reshape(E, CAPD, D)
    )
    out = np.zeros((T, D), np.float32)
    out[order[sel]] = yall[ids_s[sel], pos_s[sel]].astype(np.float32)

    # tokens beyond device capacity but within global capacity: exact host math
    ovf = (~sel) & (pos_s < CAPG)
    for t_idx in order[ovf]:
        e = expert_ids[t_idx]
        out[t_idx] = weight[e] @ x[t_idx] + bias[e]
    return out


# revision 19
# speedup vs baseline: 1.1655x; 1.0439x over previous
"""MoE grouped-linear kernel for Trainium2 (8 NeuronCores, expert-parallel).

y[t] = weight[expert_ids[t]] @ x[t] + bias[expert_ids[t]]
T=131072 tokens, E=64 experts, I=O=512, global per-expert capacity 3072
(overflow -> 0, matching the reference's capacity-bucketed dispatch).

Sharding: expert-parallel, count-adaptive. The host computes the routing
(argsort by expert), sorts experts by token count and assigns rank r to
(slot r//8, core r%8) so the 8 experts sharing a slot have similar counts;
slot k is compiled with nt[k] = ceil(max_count/128) token-tiles (the
program is built per nt-tuple and cached). Each expert's tokens are
gathered and pre-transposed on the host into the SBUF matmul layout
[128 i_lo, tile, 4 i_chunk, 128 tok_lo] fp16, so the device runs pure
dense GEMMs with no on-chip gather/scatter/transpose:

  per slot k (nt[k] token-tiles of 128):
    - one contiguous HWDGE load of X^T (SP ring; prefetched SKEW ahead;
      the first slot's load is split so matmuls start after ~0.5 MB),
    - per tile: 4 fp16 matmuls (X^T chunk stationary, W^T streaming,
      N=512) accumulate into one fp32 PSUM bank -- back-to-back warm
      matmuls at the 216 ns streaming roofline,
    - DVE evicts PSUM -> fp16 SBUF, fusing the fp32 bias add,
    - the result block is stored in ~6-tile chunks (ACT ring, separate
      from the SP load ring) so the final store tail is short.
  Weights/bias load on the ACT ring during the prologue, interleaved so
  slot 0's arrive first.

The host scatters the fp16 result blocks back to token order and upcasts
to fp32. Tokens past a slot's device capacity (pos in [2304, 3072)) are
computed exactly on the host (~never happens for uniform routing); tokens
past the global capacity 3072 are 0 like the reference.
"""

import os
import sys

sys.path.insert(0, "/opt/trn_rl_repo")

import numpy as np

T, D, E, NC = 131072, 512, 64, 8
EL = E // NC      # experts per core (= number of slots)
CAPD = 2304       # max device per-expert capacity (18 tiles of 128)
NTMAX = CAPD // 128
CAPG = 3072       # reference global per-expert capacity
SKEW = 3          # x prefetch depth (slots)
P = 128

_cache = {}
last_result = None


def _build_program(nt_slot):
    from concourse import bacc, mybir, tile

    f32 = mybir.dt.float32
    f16 = mybir.dt.float16
    ntot = sum(nt_slot)
    off = [0]
    for nt in nt_slot:
        off.append(off[-1] + nt)

    nc = bacc.Bacc(
        "TRN2",
        target_bir_lowering=False,
        debug=False,
        enable_asserts=False,
        num_devices=NC,
    )
    x_d = nc.dram_tensor("x", [P, ntot * 512], f16, kind="ExternalInput").ap()
    w_d = nc.dram_tensor("wt", [P, EL * 4 * D], f16, kind="ExternalInput").ap()
    b_d = nc.dram_tensor("bias", [1, EL * D], f16, kind="ExternalInput").ap()
    y_d = nc.dram_tensor("y", [P, ntot * 512], f16, kind="ExternalOutput").ap()

    with tile.TileContext(nc) as tc:
        with (
            tc.tile_pool(name="const", bufs=1) as constp,
            tc.tile_pool(name="wt", bufs=3) as wtp,
            tc.tile_pool(name="bt", bufs=EL) as btp,
            tc.tile_pool(name="xg0", bufs=1) as xg0p,
            tc.tile_pool(name="xg", bufs=SKEW + 1) as xgp,
            tc.tile_pool(name="ys", bufs=8) as ysp,
            tc.tile_pool(name="psY", bufs=8, space="PSUM") as psYp,
        ):
            def load_x(k):
                nt = nt_slot[k]
                if k == 0:
                    # split so the first matmuls wait on ~0.75 MB, not 2.25 MB
                    n0 = min(6, nt)
                    ta = xg0p.tile([P, n0 * 512], f16, tag="xga")
                    nc.sync.dma_start(out=ta[:], in_=x_d[:, : n0 * 512])
                    segs = [(ta, 0, n0)]
                    if nt > n0:
                        tb = xg0p.tile([P, (nt - n0) * 512], f16, tag="xgb")
                        nc.sync.dma_start(
                            out=tb[:], in_=x_d[:, n0 * 512 : nt * 512]
                        )
                        segs.append((tb, n0, nt - n0))
                    return segs
                t = xgp.tile([P, NTMAX * 512], f16, tag="xg")
                nc.sync.dma_start(
                    out=t[:, : nt * 512],
                    in_=x_d[:, off[k] * 512 : (off[k] + nt) * 512],
                )
                return [(t, 0, nt)]

            # prologue. SP ring: slot-0 weights first (the ACT ring gets
            # ~1/4 of the bandwidth while the big x loads stream), then the
            # split slot-0 x load, then x prefetch. ACT ring: raw bias
            # (16 KB), remaining weights just-in-time, then the y stores.
            # All 8 bias replicates (K=1 matmuls off the tiny raw-bias
            # tile) run up front: they fill the dead window while x/w
            # stream in and pre-warm the PE's HAM clock gate.
            ones_t = constp.tile([1, P], f16)
            nc.gpsimd.memset(ones_t[:], 1.0)
            braw = constp.tile([1, EL * D], f16)
            nc.scalar.dma_start(out=braw[:], in_=b_d)

            WSKEW = 3

            def load_w(k, eng=None):
                w = wtp.tile([P, 4 * D], f16, tag="wt")
                (eng or nc.scalar).dma_start(
                    out=w[:], in_=w_d[:, k * 4 * D : (k + 1) * 4 * D]
                )
                return w

            wpend = [load_w(0, eng=nc.sync)]
            pend = [load_x(0)]
            for k in range(1, SKEW):
                pend.append(load_x(k))
            for k in range(1, WSKEW):
                wpend.append(load_w(k))

            bts = []
            for k in range(EL):
                psB = psYp.tile([P, D], f32, tag="psY")
                nc.tensor.matmul(
                    out=psB[:],
                    lhsT=ones_t[:],
                    rhs=braw[:, k * D : (k + 1) * D],
                    start=True,
                    stop=True,
                )
                b_k = btp.tile([P, D], f32, tag="bt")
                nc.vector.tensor_copy(out=b_k[:], in_=psB[:])
                bts.append(b_k)

            for k in range(EL):
                segs = pend.pop(0)
                w_k = wpend.pop(0)
                b_k = bts[k]
                nt = nt_slot[k]
                # store-chunk boundaries; short final chunks on the last
                # slot so the kernel-tail store is small
                if k == EL - 1 and nt >= 14:
                    bnds = [6, 12, nt - 2, nt]
                else:
                    bnds = list(range(6, nt, 6)) + [nt]
                bset = set(bnds)
                ys = None
                done = 0
                for xt_t, bt0, nbt in segs:
                    for bi in range(nbt):
                        bt = bt0 + bi
                        if ys is None:
                            nxt = min(b for b in bnds if b > bt)
                            ys = ysp.tile([P, (nxt - bt) * D], f16, tag="ys")
                        psY = psYp.tile([P, D], f32, tag="psY")
                        for j in range(4):
                            nc.tensor.matmul(
                                out=psY[:],
                                lhsT=xt_t[:, bi * 512 + j * P : bi * 512 + (j + 1) * P],
                                rhs=w_k[:, j * D : (j + 1) * D],
                                start=(j == 0),
                                stop=(j == 3),
                            )
                        nc.vector.tensor_add(
                            out=ys[:, (bt - done) * D : (bt - done + 1) * D],
                            in0=psY[:],
                            in1=b_k[:],
                        )
                        if bt + 1 in bset:
                            nc.scalar.dma_start(
                                out=y_d[:, (off[k] + done) * 512 : (off[k] + bt + 1) * 512],
                                in_=ys[:, : (bt + 1 - done) * D],
                            )
                            done = bt + 1
                            ys = None
                if k + SKEW < EL:
                    pend.append(load_x(k + SKEW))
                if k + WSKEW < EL:
                    wpend.append(load_w(k + WSKEW))
    nc.compile()
    return nc


def _ensure_ntff_hook():
    """The agent image's antenv lacks axon_hooks; shim it and install the
    ctypes NTFF profiling hook so trace=True works under axon."""
    import types

    try:
        from antenv import axon_hooks  # noqa: F401
        return
    except ImportError:
        pass
    mod = types.ModuleType("antenv.axon_hooks")
    _h = {"hook": None}
    mod.set_axon_ntff_profile_hook = lambda h: _h.update(hook=h)
    mod.get_axon_ntff_profile_hook = lambda: _h["hook"]
    sys.modules["antenv.axon_hooks"] = mod
    import antenv

    antenv.axon_hooks = mod
    try:
        if "/root/.axon_site" not in sys.path:
            sys.path.insert(0, "/root/.axon_site")
        from trn_agent_boot.trn_boot import _ntff_profile_via_ctypes

        hook = _ntff_profile_via_ctypes("/opt/axon/libaxon_pjrt.so")
        if hook is not None:
            mod.set_axon_ntff_profile_hook(hook)
    except Exception:
        pass


def kernel(x, weight, bias, expert_ids):
    global last_result
    from concourse import bass_utils
    from concourse.bass_utils import run_bass_kernel_spmd

    x = np.asarray(x, dtype=np.float32)
    weight = np.asarray(weight, dtype=np.float32)
    bias = np.asarray(bias, dtype=np.float32)
    expert_ids = np.asarray(expert_ids, dtype=np.int32)

    # ---- host routing: tokens sorted by expert, position within expert ----
    order = np.argsort(expert_ids, kind="stable")
    ids_s = expert_ids[order]
    counts = np.bincount(expert_ids, minlength=E)
    starts = np.cumsum(counts) - counts
    pos_s = np.arange(T, dtype=np.int64) - starts[ids_s]
    sel = pos_s < CAPD  # tokens the device computes

    # sort experts by count desc; rank r -> (slot r//NC, core r%NC)
    counts_c = np.minimum(counts, CAPD)
    rank = np.argsort(-counts_c, kind="stable")
    perm = rank.reshape(EL, NC)  # perm[slot, core] = expert id
    nt_slot = tuple(
        max(1, int(-(-counts_c[perm[k]].max() // 128))) for k in range(EL)
    )
    off = [0]
    for nt in nt_slot:
        off.append(off[-1] + nt)
    ntot = off[-1]

    if nt_slot not in _cache:
        _cache[nt_slot] = _build_program(nt_slot)
    nc = _cache[nt_slot]

    # ---- pack x: [E, CAPD, D] fp16, then to [E, 128 i_lo, bt, j, 128 t_lo] ----
    x16 = x.astype(np.float16)
    buf = np.zeros((E, CAPD, D), np.float16)
    buf[ids_s[sel], pos_s[sel]] = x16[order[sel]]
    xt = np.ascontiguousarray(
        buf.reshape(E, NTMAX, P, 4, P).transpose(0, 4, 1, 3, 2)
    ).reshape(E, P, NTMAX * 512)

    # ---- weights: [E, O, I] -> W^T tile layout [E, 128 i_lo, 4 j * 512 o] ----
    wt16 = np.ascontiguousarray(weight.transpose(0, 2, 1)).astype(np.float16)
    wt16 = np.ascontiguousarray(
        wt16.reshape(E, 4, P, D).transpose(0, 2, 1, 3)
    ).reshape(E, P, 4 * D)

    in_maps = []
    for c in range(NC):
        ex = perm[:, c]
        in_maps.append(
            {
                "x": np.concatenate(
                    [xt[ex[k]][:, : nt_slot[k] * 512] for k in range(EL)], axis=1
                ),
                "wt": np.ascontiguousarray(
                    wt16[ex].transpose(1, 0, 2).reshape(P, EL * 4 * D)
                ),
                "bias": np.ascontiguousarray(
                    bias[ex].reshape(1, EL * D).astype(np.float16)
                ),
            }
        )

    trace = bool(int(os.environ.get("KERNEL_TRACE", "0")))
    kwargs = {}
    if trace:
        _ensure_ntff_hook()
        bass_utils.upload_artifacts = lambda tmpdir: "local://" + tmpdir
        tdir = os.environ.get("KERNEL_TRACE_DIR")
        if tdir:
            os.makedirs(tdir, exist_ok=True)
            kwargs["tmpdir"] = tdir
    res = run_bass_kernel_spmd(
        nc, in_maps, core_ids=list(range(NC)), trace=trace, **kwargs
    )
    last_result = res

    # ---- unpack: y blocks [128 t_lo, bt*512+o] per (slot, core) -> [E, CAPD, D]
    ypad = np.zeros((E, P, NTMAX * 512), np.float16)
    for c in range(NC):
        yc = res.results[c]["y"]
        for k in range(EL):
            ypad[perm[k, c]][:, : nt_slot[k] * 512] = yc[
                :, off[k] * 512 : (off[k] + nt_slot[k]) * 512
            ]
    yall = (
        ypad.reshape(E, P, NTMAX, D).transpose(0, 2, 1, 3).reshape(E, CAPD, D)
    )
    out = np.zeros((T, D), np.float32)
    out[order[sel]] = yall[ids_s[sel], pos_s[sel]].astype(np.float32)

    # tokens beyond device capacity but within global capacity: exact host math
    ovf = (~sel) & (pos_s < CAPG)
    for t_idx in order[ovf]:
        e = expert_ids[t_idx]
        out[t_idx] = weight[e] @ x[t_idx] + bias[e]
    return out


# revision 20
# speedup vs baseline: 1.1665x; 1.0009x over previous
"""MoE grouped-linear kernel for Trainium2 (8 NeuronCores, expert-parallel).

y[t] = weight[expert_ids[t]] @ x[t] + bias[expert_ids[t]]
T=131072 tokens, E=64 experts, I=O=512, global per-expert capacity 3072
(overflow -> 0, matching the reference's capacity-bucketed dispatch).

Sharding: expert-parallel, count-adaptive. The host computes the routing
(argsort by expert), sorts experts by token count and assigns rank r to
(slot r//8, core r%8) so the 8 experts sharing a slot have similar counts;
slot k is compiled with nt[k] = ceil(max_count/128) token-tiles (the
program is built per nt-tuple and cached). Each expert's tokens are
gathered and pre-transposed on the host into the SBUF matmul layout
[128 i_lo, tile, 4 i_chunk, 128 tok_lo] fp16, so the device runs pure
dense GEMMs with no on-chip gather/scatter/transpose:

  per slot k (nt[k] token-tiles of 128):
    - one contiguous HWDGE load of X^T (SP ring; prefetched SKEW ahead;
      the first slot's load is split so matmuls start after ~0.5 MB),
    - per tile: 4 fp16 matmuls (X^T chunk stationary, W^T streaming,
      N=512) accumulate into one fp32 PSUM bank -- back-to-back warm
      matmuls at the 216 ns streaming roofline,
    - DVE evicts PSUM -> fp16 SBUF, fusing the fp32 bias add,
    - the result block is stored in ~6-tile chunks (ACT ring, separate
      from the SP load ring) so the final store tail is short.
  Weights/bias load on the ACT ring during the prologue, interleaved so
  slot 0's arrive first.

The host scatters the fp16 result blocks back to token order and upcasts
to fp32. Tokens past a slot's device capacity (pos in [2304, 3072)) are
computed exactly on the host (~never happens for uniform routing); tokens
past the global capacity 3072 are 0 like the reference.
"""

import os
import sys

sys.path.insert(0, "/opt/trn_rl_repo")

import numpy as np

T, D, E, NC = 131072, 512, 64, 8
EL = E // NC      # experts per core (= number of slots)
CAPD = 2304       # max device per-expert capacity (18 tiles of 128)
NTMAX = CAPD // 128
CAPG = 3072       # reference global per-expert capacity
SKEW = 3          # x prefetch depth (slots)
P = 128

_cache = {}
last_result = None


def _build_program(nt_slot):
    from concourse import bacc, mybir, tile

    f32 = mybir.dt.float32
    f16 = mybir.dt.float16
    ntot = sum(nt_slot)
    off = [0]
    for nt in nt_slot:
        off.append(off[-1] + nt)

    nc = bacc.Bacc(
        "TRN2",
        target_bir_lowering=False,
        debug=False,
        enable_asserts=False,
        num_devices=NC,
    )
    x_d = nc.dram_tensor("x", [P, ntot * 512], f16, kind="ExternalInput").ap()
    w_d = nc.dram_tensor("wt", [P, EL * 4 * D], f16, kind="ExternalInput").ap()
    b_d = nc.dram_tensor("bias", [1, EL * D], f16, kind="ExternalInput").ap()
    y_d = nc.dram_tensor("y", [P, ntot * 512], f16, kind="ExternalOutput").ap()

    with tile.TileContext(nc) as tc:
        with (
            tc.tile_pool(name="const", bufs=1) as constp,
            tc.tile_pool(name="wt", bufs=3) as wtp,
            tc.tile_pool(name="bt", bufs=EL) as btp,
            tc.tile_pool(name="xg0", bufs=1) as xg0p,
            tc.tile_pool(name="xg", bufs=SKEW + 1) as xgp,
            tc.tile_pool(name="ys", bufs=8) as ysp,
            tc.tile_pool(name="psY", bufs=8, space="PSUM") as psYp,
        ):
            def load_x(k):
                nt = nt_slot[k]
                if k == 0:
                    # split so the first matmuls wait on ~0.75 MB, not 2.25 MB
                    n0 = min(6, nt)
                    ta = xg0p.tile([P, n0 * 512], f16, tag="xga")
                    nc.sync.dma_start(out=ta[:], in_=x_d[:, : n0 * 512])
                    segs = [(ta, 0, n0)]
                    if nt > n0:
                        tb = xg0p.tile([P, (nt - n0) * 512], f16, tag="xgb")
                        nc.sync.dma_start(
                            out=tb[:], in_=x_d[:, n0 * 512 : nt * 512]
                        )
                        segs.append((tb, n0, nt - n0))
                    return segs
                t = xgp.tile([P, NTMAX * 512], f16, tag="xg")
                nc.sync.dma_start(
                    out=t[:, : nt * 512],
                    in_=x_d[:, off[k] * 512 : (off[k] + nt) * 512],
                )
                return [(t, 0, nt)]

            # prologue. SP ring: slot-0 weights first (the ACT ring gets
            # ~1/4 of the bandwidth while the big x loads stream), then the
            # split slot-0 x load, then x prefetch. ACT ring: raw bias
            # (16 KB), remaining weights just-in-time, then the y stores.
            # All 8 bias replicates (K=1 matmuls off the tiny raw-bias
            # tile) run up front: they fill the dead window while x/w
            # stream in and pre-warm the PE's HAM clock gate.
            ones_t = constp.tile([1, P], f16)
            nc.gpsimd.memset(ones_t[:], 1.0)
            braw = constp.tile([1, EL * D], f16)
            nc.sync.dma_start(out=braw[:], in_=b_d)

            WSKEW = 3

            def load_w(k, eng=None):
                w = wtp.tile([P, 4 * D], f16, tag="wt")
                (eng or nc.scalar).dma_start(
                    out=w[:], in_=w_d[:, k * 4 * D : (k + 1) * 4 * D]
                )
                return w

            wpend = [load_w(0, eng=nc.sync)]
            pend = [load_x(0)]
            for k in range(1, SKEW):
                pend.append(load_x(k))
            for k in range(1, WSKEW):
                wpend.append(load_w(k))

            bts = []
            for k in range(EL):
                psB = psYp.tile([P, D], f32, tag="psY")
                nc.tensor.matmul(
                    out=psB[:],
                    lhsT=ones_t[:],
                    rhs=braw[:, k * D : (k + 1) * D],
                    start=True,
                    stop=True,
                )
                b_k = btp.tile([P, D], f32, tag="bt")
                nc.vector.tensor_copy(out=b_k[:], in_=psB[:])
                bts.append(b_k)

            for k in range(EL):
                segs = pend.pop(0)
                w_k = wpend.pop(0)
                b_k = bts[k]
                nt = nt_slot[k]
                # store-chunk boundaries; short final chunks on the last
                # slot so the kernel-tail store is small
                if k == EL - 1 and nt >= 14:
                    bnds = [6, 12, nt - 2, nt]
                else:
                    bnds = list(range(6, nt, 6)) + [nt]
                bset = set(bnds)
                ys = None
                done = 0
                for xt_t, bt0, nbt in segs:
                    for bi in range(nbt):
                        bt = bt0 + bi
                        if ys is None:
                            nxt = min(b for b in bnds if b > bt)
                            ys = ysp.tile([P, (nxt - bt) * D], f16, tag="ys")
                        psY = psYp.tile([P, D], f32, tag="psY")
                        for j in range(4):
                            nc.tensor.matmul(
                                out=psY[:],
                                lhsT=xt_t[:, bi * 512 + j * P : bi * 512 + (j + 1) * P],
                                rhs=w_k[:, j * D : (j + 1) * D],
                                start=(j == 0),
                                stop=(j == 3),
                            )
                        nc.vector.tensor_add(
                            out=ys[:, (bt - done) * D : (bt - done + 1) * D],
                            in0=psY[:],
                            in1=b_k[:],
                        )
                        if bt + 1 in bset:
                            nc.scalar.dma_start(
                                out=y_d[:, (off[k] + done) * 512 : (off[k] + bt + 1) * 512],
                                in_=ys[:, : (bt + 1 - done) * D],
                            )
                            done = bt + 1
                            ys = None
                if k + SKEW < EL:
                    pend.append(load_x(k + SKEW))
                if k + WSKEW < EL:
                    wpend.append(load_w(k + WSKEW))
    nc.compile()
    return nc


def _ensure_ntff_hook():
    """The agent image's antenv lacks axon_hooks; shim it and install the
    ctypes NTFF profiling hook so trace=True works under axon."""
    import types

    try:
        from antenv import axon_hooks  # noqa: F401
        return
    except ImportError:
        pass
    mod = types.ModuleType("antenv.axon_hooks")
    _h = {"hook": None}
    mod.set_axon_ntff_profile_hook = lambda h: _h.update(hook=h)
    mod.get_axon_ntff_profile_hook = lambda: _h["hook"]
    sys.modules["antenv.axon_hooks"] = mod
    import antenv

    antenv.axon_hooks = mod
    try:
        if "/root/.axon_site" not in sys.path:
            sys.path.insert(0, "/root/.axon_site")
        from trn_agent_boot.trn_boot import _ntff_profile_via_ctypes

        hook = _ntff_profile_via_ctypes("/opt/axon/libaxon_pjrt.so")
        if hook is not None:
            mod.set_axon_ntff_profile_hook(hook)
    except Exception:
        pass


def kernel(x, weight, bias, expert_ids):
    global last_result
    from concourse import bass_utils
    from concourse.bass_utils import run_bass_kernel_spmd

    x = np.asarray(x, dtype=np.float32)
    weight = np.asarray(weight, dtype=np.float32)
    bias = np.asarray(bias, dtype=np.float32)
    expert_ids = np.asarray(expert_ids, dtype=np.int32)

    # ---- host routing: tokens sorted by expert, position within expert ----
    order = np.argsort(expert_ids, kind="stable")
    ids_s = expert_ids[order]
    counts = np.bincount(expert_ids, minlength=E)
    starts = np.cumsum(counts) - counts
    pos_s = np.arange(T, dtype=np.int64) - starts[ids_s]
    sel = pos_s < CAPD  # tokens the device computes

    # sort experts by count desc; rank r -> (slot r//NC, core r%NC)
    counts_c = np.minimum(counts, CAPD)
    rank = np.argsort(-counts_c, kind="stable")
    perm = rank.reshape(EL, NC)  # perm[slot, core] = expert id
    nt_slot = tuple(
        max(1, int(-(-counts_c[perm[k]].max() // 128))) for k in range(EL)
    )
    off = [0]
    for nt in nt_slot:
        off.append(off[-1] + nt)
    ntot = off[-1]

    if nt_slot not in _cache:
        _cache[nt_slot] = _build_program(nt_slot)
    nc = _cache[nt_slot]

    # ---- pack x: [E, CAPD, D] fp16, then to [E, 128 i_lo, bt, j, 128 t_lo] ----
    x16 = x.astype(np.float16)
    buf = np.zeros((E, CAPD, D), np.float16)
    buf[ids_s[sel], pos_s[sel]] = x16[order[sel]]
    xt = np.ascontiguousarray(
        buf.reshape(E, NTMAX, P, 4, P).transpose(0, 4, 1, 3, 2)
    ).reshape(E, P, NTMAX * 512)

    # ---- weights: [E, O, I] -> W^T tile layout [E, 128 i_lo, 4 j * 512 o] ----
    wt16 = np.ascontiguousarray(weight.transpose(0, 2, 1)).astype(np.float16)
    wt16 = np.ascontiguousarray(
        wt16.reshape(E, 4, P, D).transpose(0, 2, 1, 3)
    ).reshape(E, P, 4 * D)

    in_maps = []
    for c in range(NC):
        ex = perm[:, c]
        in_maps.append(
            {
                "x": np.concatenate(
                    [xt[ex[k]][:, : nt_slot[k] * 512] for k in range(EL)], axis=1
                ),
                "wt": np.ascontiguousarray(
                    wt16[ex].transpose(1, 0, 2).reshape(P, EL * 4 * D)
                ),
                "bias": np.ascontiguousarray(
                    bias[ex].reshape(1, EL * D).astype(np.float16)
                ),
            }
        )

    trace = bool(int(os.environ.get("KERNEL_TRACE", "0")))
    kwargs = {}
    if trace:
        _ensure_ntff_hook()
        bass_utils.upload_artifacts = lambda tmpdir: "local://" + tmpdir
        tdir = os.environ.get("KERNEL_TRACE_DIR")
        if tdir:
            os.makedirs(tdir, exist_ok=True)
            kwargs["tmpdir"] = tdir
    res = run_bass_kernel_spmd(
        nc, in_maps, core_ids=list(range(NC)), trace=trace, **kwargs
    )
    last_result = res

    # ---- unpack: y blocks [128 t_lo, bt*512+o] per (slot, core) -> [E, CAPD, D]
    ypad = np.zeros((E, P, NTMAX * 512), np.float16)
    for c in range(NC):
        yc = res.results[c]["y"]
        for k in range(EL):
            ypad[perm[k, c]][:, : nt_slot[k] * 512] = yc[
                :, off[k] * 512 : (off[k] + nt_slot[k]) * 512
            ]
    yall = (
        ypad.reshape(E, P, NTMAX, D).transpose(0, 2, 1, 3).reshape(E, CAPD, D)
    )
    out = np.zeros((T, D), np.float32)
    out[order[sel]] = yall[ids_s[sel], pos_s[sel]].astype(np.float32)

    # tokens beyond device capacity but within global capacity: exact host math
    ovf = (~sel) & (pos_s < CAPG)
    for t_idx in order[ovf]:
        e = expert_ids[t_idx]
        out[t_idx] = weight[e] @ x[t_idx] + bias[e]
    return out


# revision 22
# speedup vs baseline: 1.1721x; 1.0048x over previous
"""MoE grouped-linear kernel for Trainium2 (8 NeuronCores, expert-parallel).

y[t] = weight[expert_ids[t]] @ x[t] + bias[expert_ids[t]]
T=131072 tokens, E=64 experts, I=O=512, global per-expert capacity 3072
(overflow -> 0, matching the reference's capacity-bucketed dispatch).

Sharding: expert-parallel, count-adaptive. The host computes the routing
(argsort by expert), sorts experts by token count and assigns rank r to
(slot r//8, core r%8) so the 8 experts sharing a slot have similar counts;
slot k is compiled with nt[k] = ceil(max_count/128) token-tiles (the
program is built per nt-tuple and cached). Each expert's tokens are
gathered and pre-transposed on the host into the SBUF matmul layout
[128 i_lo, tile, 4 i_chunk, 128 tok_lo] fp16, so the device runs pure
dense GEMMs with no on-chip gather/scatter/transpose:

  per slot k (nt[k] token-tiles of 128):
    - one contiguous HWDGE load of X^T (SP ring; prefetched SKEW ahead;
      the first slot's load is split so matmuls start after ~0.5 MB),
    - per tile: 4 fp16 matmuls (X^T chunk stationary, W^T streaming,
      N=512) accumulate into one fp32 PSUM bank -- back-to-back warm
      matmuls at the 216 ns streaming roofline,
    - DVE evicts PSUM -> fp16 SBUF, fusing the fp32 bias add,
    - the result block is stored in ~6-tile chunks (ACT ring, separate
      from the SP load ring) so the final store tail is short.
  Weights/bias load on the ACT ring during the prologue, interleaved so
  slot 0's arrive first.

The host scatters the fp16 result blocks back to token order and upcasts
to fp32. Tokens past a slot's device capacity (pos in [2304, 3072)) are
computed exactly on the host (~never happens for uniform routing); tokens
past the global capacity 3072 are 0 like the reference.
"""

import os
import sys

sys.path.insert(0, "/opt/trn_rl_repo")

import numpy as np

T, D, E, NC = 131072, 512, 64, 8
EL = E // NC      # experts per core (= number of slots)
CAPD = 2304       # max device per-expert capacity (18 tiles of 128)
NTMAX = CAPD // 128
CAPG = 3072       # reference global per-expert capacity
SKEW = 3          # x prefetch depth (slots)
P = 128

_cache = {}
last_result = None


def _build_program(nt_slot):
    from concourse import bacc, mybir, tile

    f32 = mybir.dt.float32
    f16 = mybir.dt.float16
    ntot = sum(nt_slot)
    off = [0]
    for nt in nt_slot:
        off.append(off[-1] + nt)

    nc = bacc.Bacc(
        "TRN2",
        target_bir_lowering=False,
        debug=False,
        enable_asserts=False,
        num_devices=NC,
    )
    x_d = nc.dram_tensor("x", [P, ntot * 512], f16, kind="ExternalInput").ap()
    w_d = nc.dram_tensor("wt", [P, EL * 4 * D], f16, kind="ExternalInput").ap()
    b_d = nc.dram_tensor("bias", [1, EL * D], f16, kind="ExternalInput").ap()
    y_d = nc.dram_tensor("y", [P, ntot * 512], f16, kind="ExternalOutput").ap()

    with tile.TileContext(nc) as tc:
        with (
            tc.tile_pool(name="const", bufs=1) as constp,
            tc.tile_pool(name="wt", bufs=3) as wtp,
            tc.tile_pool(name="bt", bufs=EL) as btp,
            tc.tile_pool(name="xg0", bufs=1) as xg0p,
            tc.tile_pool(name="xg", bufs=SKEW + 1) as xgp,
            tc.tile_pool(name="ys", bufs=8) as ysp,
            tc.tile_pool(name="psY", bufs=8, space="PSUM") as psYp,
        ):
            def load_x(k):
                nt = nt_slot[k]
                if k == 0:
                    # split so the first matmuls wait on ~0.75 MB, not 2.25 MB
                    n0 = min(6, nt)
                    ta = xg0p.tile([P, n0 * 512], f16, tag="xga")
                    nc.sync.dma_start(out=ta[:], in_=x_d[:, : n0 * 512])
                    segs = [(ta, 0, n0)]
                    if nt > n0:
                        tb = xg0p.tile([P, (nt - n0) * 512], f16, tag="xgb")
                        nc.sync.dma_start(
                            out=tb[:], in_=x_d[:, n0 * 512 : nt * 512]
                        )
                        segs.append((tb, n0, nt - n0))
                    return segs
                t = xgp.tile([P, NTMAX * 512], f16, tag="xg")
                nc.sync.dma_start(
                    out=t[:, : nt * 512],
                    in_=x_d[:, off[k] * 512 : (off[k] + nt) * 512],
                )
                return [(t, 0, nt)]

            # prologue. SP ring: slot-0 weights first (the ACT ring gets
            # ~1/4 of the bandwidth while the big x loads stream), then the
            # split slot-0 x load, then x prefetch. ACT ring: raw bias
            # (16 KB), remaining weights just-in-time, then the y stores.
            # All 8 bias replicates (K=1 matmuls off the tiny raw-bias
            # tile) run up front: they fill the dead window while x/w
            # stream in and pre-warm the PE's HAM clock gate.
            # 0.5-valued ones: each bias replicate is TWO accumulating K=1
            # matmuls (0.5b + 0.5b, exact in fp32) — doubled purely to
            # stretch the PE warmup that bridges the initial x/w DMA wait
            # and flips the HAM clock gate before the tile matmuls start.
            ones_t = constp.tile([1, P], f16)
            nc.gpsimd.memset(ones_t[:], 0.5)
            braw = constp.tile([1, EL * D], f16)
            nc.sync.dma_start(out=braw[:], in_=b_d)

            WSKEW = 3

            def load_w(k, eng=None):
                w = wtp.tile([P, 4 * D], f16, tag="wt")
                (eng or nc.scalar).dma_start(
                    out=w[:], in_=w_d[:, k * 4 * D : (k + 1) * 4 * D]
                )
                return w

            wpend = [load_w(0, eng=nc.sync)]
            pend = [load_x(0)]
            for k in range(1, SKEW):
                pend.append(load_x(k))
            for k in range(1, WSKEW):
                wpend.append(load_w(k))

            bts = []
            for k in range(EL):
                psB = psYp.tile([P, D], f32, tag="psY")
                for r in range(2):
                    nc.tensor.matmul(
                        out=psB[:],
                        lhsT=ones_t[:],
                        rhs=braw[:, k * D : (k + 1) * D],
                        start=(r == 0),
                        stop=(r == 1),
                    )
                b_k = btp.tile([P, D], f32, tag="bt")
                nc.vector.tensor_copy(out=b_k[:], in_=psB[:])
                bts.append(b_k)

            for k in range(EL):
                segs = pend.pop(0)
                w_k = wpend.pop(0)
                b_k = bts[k]
                nt = nt_slot[k]
                # store-chunk boundaries; short final chunks on the last
                # slot so the kernel-tail store is small
                if k == EL - 1 and nt >= 14:
                    bnds = [6, 12, nt - 2, nt]
                else:
                    bnds = list(range(6, nt, 6)) + [nt]
                bset = set(bnds)
                ys = None
                done = 0
                for xt_t, bt0, nbt in segs:
                    for bi in range(nbt):
                        bt = bt0 + bi
                        if ys is None:
                            nxt = min(b for b in bnds if b > bt)
                            ys = ysp.tile([P, (nxt - bt) * D], f16, tag="ys")
                        psY = psYp.tile([P, D], f32, tag="psY")
                        for j in range(4):
                            nc.tensor.matmul(
                                out=psY[:],
                                lhsT=xt_t[:, bi * 512 + j * P : bi * 512 + (j + 1) * P],
                                rhs=w_k[:, j * D : (j + 1) * D],
                                start=(j == 0),
                                stop=(j == 3),
                            )
                        nc.vector.tensor_add(
                            out=ys[:, (bt - done) * D : (bt - done + 1) * D],
                            in0=psY[:],
                            in1=b_k[:],
                        )
                        if bt + 1 in bset:
                            nc.scalar.dma_start(
                                out=y_d[:, (off[k] + done) * 512 : (off[k] + bt + 1) * 512],
                                in_=ys[:, : (bt + 1 - done) * D],
                            )
                            done = bt + 1
                            ys = None
                if k + SKEW < EL:
                    pend.append(load_x(k + SKEW))
                if k + WSKEW < EL:
                    wpend.append(load_w(k + WSKEW))
    nc.compile()
    return nc


def _ensure_ntff_hook():
    """The agent image's antenv lacks axon_hooks; shim it and install the
    ctypes NTFF profiling hook so trace=True works under axon."""
    import types

    try:
        from antenv import axon_hooks  # noqa: F401
        return
    except ImportError:
        pass
    mod = types.ModuleType("antenv.axon_hooks")
    _h = {"hook": None}
    mod.set_axon_ntff_profile_hook = lambda h: _h.update(hook=h)
    mod.get_axon_ntff_profile_hook = lambda: _h["hook"]
    sys.modules["antenv.axon_hooks"] = mod
    import antenv

    antenv.axon_hooks = mod
    try:
        if "/root/.axon_site" not in sys.path:
            sys.path.insert(0, "/root/.axon_site")
        from trn_agent_boot.trn_boot import _ntff_profile_via_ctypes

        hook = _ntff_profile_via_ctypes("/opt/axon/libaxon_pjrt.so")
        if hook is not None:
            mod.set_axon_ntff_profile_hook(hook)
    except Exception:
        pass


def kernel(x, weight, bias, expert_ids):
    global last_result
    from concourse import bass_utils
    from concourse.bass_utils import run_bass_kernel_spmd

    x = np.asarray(x, dtype=np.float32)
    weight = np.asarray(weight, dtype=np.float32)
    bias = np.asarray(bias, dtype=np.float32)
    expert_ids = np.asarray(expert_ids, dtype=np.int32)

    # ---- host routing: tokens sorted by expert, position within expert ----
    order = np.argsort(expert_ids, kind="stable")
    ids_s = expert_ids[order]
    counts = np.bincount(expert_ids, minlength=E)
    starts = np.cumsum(counts) - counts
    pos_s = np.arange(T, dtype=np.int64) - starts[ids_s]
    sel = pos_s < CAPD  # tokens the device computes

    # sort experts by count desc; rank r -> (slot r//NC, core r%NC)
    counts_c = np.minimum(counts, CAPD)
    rank = np.argsort(-counts_c, kind="stable")
    perm = rank.reshape(EL, NC)  # perm[slot, core] = expert id
    nt_slot = tuple(
        max(1, int(-(-counts_c[perm[k]].max() // 128))) for k in range(EL)
    )
    off = [0]
    for nt in nt_slot:
        off.append(off[-1] + nt)
    ntot = off[-1]

    if nt_slot not in _cache:
        _cache[nt_slot] = _build_program(nt_slot)
    nc = _cache[nt_slot]

    # ---- pack x: [E, CAPD, D] fp16, then to [E, 128 i_lo, bt, j, 128 t_lo] ----
    x16 = x.astype(np.float16)
    buf = np.zeros((E, CAPD, D), np.float16)
    buf[ids_s[sel], pos_s[sel]] = x16[order[sel]]
    xt = np.ascontiguousarray(
        buf.reshape(E, NTMAX, P, 4, P).transpose(0, 4, 1, 3, 2)
    ).reshape(E, P, NTMAX * 512)

    # ---- weights: [E, O, I] -> W^T tile layout [E, 128 i_lo, 4 j * 512 o] ----
    wt16 = np.ascontiguousarray(weight.transpose(0, 2, 1)).astype(np.float16)
    wt16 = np.ascontiguousarray(
        wt16.reshape(E, 4, P, D).transpose(0, 2, 1, 3)
    ).reshape(E, P, 4 * D)

    in_maps = []
    for c in range(NC):
        ex = perm[:, c]
        in_maps.append(
            {
                "x": np.concatenate(
                    [xt[ex[k]][:, : nt_slot[k] * 512] for k in range(EL)], axis=1
                ),
                "wt": np.ascontiguousarray(
                    wt16[ex].transpose(1, 0, 2).reshape(P, EL * 4 * D)
                ),
                "bias": np.ascontiguousarray(
                    bias[ex].reshape(1, EL * D).astype(np.float16)
                ),
            }
        )

    trace = bool(int(os.environ.get("KERNEL_TRACE", "0")))
    kwargs = {}
    if trace:
        _ensure_ntff_hook()
        bass_utils.upload_artifacts = lambda tmpdir: "local://" + tmpdir
        tdir = os.environ.get("KERNEL_TRACE_DIR")
        if tdir:
            os.makedirs(tdir, exist_ok=True)
            kwargs["tmpdir"] = tdir
    res = run_bass_kernel_spmd(
        nc, in_maps, core_ids=list(range(NC)), trace=trace, **kwargs
    )
    last_result = res

    # ---- unpack: y blocks [128 t_lo, bt*512+o] per (slot, core) -> [E, CAPD, D]
    ypad = np.zeros((E, P, NTMAX * 512), np.float16)
    for c in range(NC):
        yc = res.results[c]["y"]
        for k in range(EL):
            ypad[perm[k, c]][:, : nt_slot[k] * 512] = yc[
                :, off[k] * 512 : (off[k] + nt_slot[k]) * 512
            ]
    yall = (
        ypad.reshape(E, P, NTMAX, D).transpose(0, 2, 1, 3).reshape(E, CAPD, D)
    )
    out = np.zeros((T, D), np.float32)
    out[order[sel]] = yall[ids_s[sel], pos_s[sel]].astype(np.float32)

    # tokens beyond device capacity but within global capacity: exact host math
    ovf = (~sel) & (pos_s < CAPG)
    for t_idx in order[ovf]:
        e = expert_ids[t_idx]
        out[t_idx] = weight[e] @ x[t_idx] + bias[e]
    return out


# revision 24
# speedup vs baseline: 1.1757x; 1.0030x over previous
"""MoE grouped-linear kernel for Trainium2 (8 NeuronCores, expert-parallel).

y[t] = weight[expert_ids[t]] @ x[t] + bias[expert_ids[t]]
T=131072 tokens, E=64 experts, I=O=512, global per-expert capacity 3072
(overflow -> 0, matching the reference's capacity-bucketed dispatch).

Sharding: expert-parallel, count-adaptive. The host computes the routing
(argsort by expert), sorts experts by token count and assigns rank r to
(slot r//8, core r%8) so the 8 experts sharing a slot have similar counts;
slot k is compiled with nt[k] = ceil(max_count/128) token-tiles (the
program is built per nt-tuple and cached). Each expert's tokens are
gathered and pre-transposed on the host into the SBUF matmul layout
[128 i_lo, tile, 4 i_chunk, 128 tok_lo] fp16, so the device runs pure
dense GEMMs with no on-chip gather/scatter/transpose:

  per slot k (nt[k] token-tiles of 128):
    - one contiguous HWDGE load of X^T (SP ring; prefetched SKEW ahead;
      the first slot's load is split so matmuls start after ~0.75 MB),
    - per tile: 4 fp16 matmuls (X^T chunk stationary, W^T streaming,
      N=512) accumulate into one fp32 PSUM bank -- back-to-back warm
      matmuls at the 216 ns streaming roofline,
    - DVE evicts PSUM -> fp16 SBUF, fusing the fp32 bias add,
    - the result block is stored in ~6-tile chunks (ACT ring, separate
      from the SP load ring) so the final store tail is short.
  Bias is sent raw (16 KB) and replicated across partitions on-chip by
  K=1 matmuls that double as the PE warmup; slot-0 weights load at the
  head of the SP ring (the ACT ring gets ~1/4 bandwidth while big x
  descriptors stream), the rest just-in-time on the ACT ring.

The host scatters the fp16 result blocks back to token order and upcasts
to fp32. Tokens past a slot's device capacity (pos in [2304, 3072)) are
computed exactly on the host (~never happens for uniform routing); tokens
past the global capacity 3072 are 0 like the reference.
"""

import os
import sys

sys.path.insert(0, "/opt/trn_rl_repo")

import numpy as np

T, D, E, NC = 131072, 512, 64, 8
EL = E // NC      # experts per core (= number of slots)
CAPD = 2304       # max device per-expert capacity (18 tiles of 128)
NTMAX = CAPD // 128
CAPG = 3072       # reference global per-expert capacity
SKEW = 3          # x prefetch depth (slots)
P = 128

_cache = {}
last_result = None


def _build_program(nt_slot):
    from concourse import bacc, mybir, tile

    f32 = mybir.dt.float32
    f16 = mybir.dt.float16
    ntot = sum(nt_slot)
    off = [0]
    for nt in nt_slot:
        off.append(off[-1] + nt)

    nc = bacc.Bacc(
        "TRN2",
        target_bir_lowering=False,
        debug=False,
        enable_asserts=False,
        num_devices=NC,
    )
    x_d = nc.dram_tensor("x", [P, ntot * 512], f16, kind="ExternalInput").ap()
    w_d = nc.dram_tensor("wt", [P, EL * 4 * D], f16, kind="ExternalInput").ap()
    b_d = nc.dram_tensor("bias", [1, EL * D], f16, kind="ExternalInput").ap()
    y_d = nc.dram_tensor("y", [P, ntot * 512], f16, kind="ExternalOutput").ap()

    with tile.TileContext(nc) as tc:
        with (
            tc.tile_pool(name="const", bufs=1) as constp,
            tc.tile_pool(name="wt", bufs=3) as wtp,
            tc.tile_pool(name="bt", bufs=EL) as btp,
            tc.tile_pool(name="xg0", bufs=1) as xg0p,
            tc.tile_pool(name="xg", bufs=SKEW + 1) as xgp,
            tc.tile_pool(name="ys", bufs=8) as ysp,
            tc.tile_pool(name="psY", bufs=8, space="PSUM") as psYp,
        ):
            def load_x(k):
                nt = nt_slot[k]
                if k == 0:
                    # split so the first matmuls wait on ~0.75 MB, not 2.25 MB
                    n0 = min(6, nt)
                    ta = xg0p.tile([P, n0 * 512], f16, tag="xga")
                    nc.sync.dma_start(out=ta[:], in_=x_d[:, : n0 * 512])
                    segs = [(ta, 0, n0)]
                    if nt > n0:
                        tb = xg0p.tile([P, (nt - n0) * 512], f16, tag="xgb")
                        nc.sync.dma_start(
                            out=tb[:], in_=x_d[:, n0 * 512 : nt * 512]
                        )
                        segs.append((tb, n0, nt - n0))
                    return segs
                t = xgp.tile([P, NTMAX * 512], f16, tag="xg")
                nc.sync.dma_start(
                    out=t[:, : nt * 512],
                    in_=x_d[:, off[k] * 512 : (off[k] + nt) * 512],
                )
                return [(t, 0, nt)]

            # prologue. SP ring: raw bias (16 KB), slot-0 weights (the ACT
            # ring gets ~1/4 of the bandwidth while big x loads stream),
            # the split slot-0 x load, then x prefetch. ACT ring: the
            # remaining weights just-in-time, then the y stores. All bias
            # replicates (K=1 matmuls off the raw-bias tile) run up front,
            # filling the dead window while x/w stream in.
            # 0.5-valued ones: each bias replicate is TWO accumulating K=1
            # matmuls (0.5b + 0.5b, exact in fp32) — doubled purely to
            # stretch the PE warmup that bridges the initial x/w DMA wait
            # and flips the HAM clock gate before the tile matmuls start.
            ones_t = constp.tile([1, P], f16)
            nc.gpsimd.memset(ones_t[:], 0.5)
            braw = constp.tile([1, EL * D], f16)
            nc.sync.dma_start(out=braw[:], in_=b_d)

            WSKEW = 3

            def load_w(k, eng=None):
                w = wtp.tile([P, 4 * D], f16, tag="wt")
                (eng or nc.scalar).dma_start(
                    out=w[:], in_=w_d[:, k * 4 * D : (k + 1) * 4 * D]
                )
                return w

            wpend = [load_w(0, eng=nc.sync)]
            pend = [load_x(0)]
            for k in range(1, SKEW):
                pend.append(load_x(k))
            for k in range(1, WSKEW):
                wpend.append(load_w(k))

            bts = []
            for k in range(EL):
                psB = psYp.tile([P, D], f32, tag="psY")
                for r in range(2):
                    nc.tensor.matmul(
                        out=psB[:],
                        lhsT=ones_t[:],
                        rhs=braw[:, k * D : (k + 1) * D],
                        start=(r == 0),
                        stop=(r == 1),
                    )
                b_k = btp.tile([P, D], f32, tag="bt")
                nc.vector.tensor_copy(out=b_k[:], in_=psB[:])
                bts.append(b_k)

            for k in range(EL):
                segs = pend.pop(0)
                w_k = wpend.pop(0)
                b_k = bts[k]
                nt = nt_slot[k]
                # store-chunk boundaries; short final chunks on the last
                # slot so the kernel-tail store is small
                if k == EL - 1 and nt >= 14:
                    bnds = [6, 12, nt - 2, nt]
                else:
                    bnds = list(range(6, nt, 6)) + [nt]
                bset = set(bnds)
                ys = None
                done = 0
                for xt_t, bt0, nbt in segs:
                    for bi in range(nbt):
                        bt = bt0 + bi
                        if ys is None:
                            nxt = min(b for b in bnds if b > bt)
                            ys = ysp.tile([P, (nxt - bt) * D], f16, tag="ys")
                        psY = psYp.tile([P, D], f32, tag="psY")
                        for j in range(4):
                            nc.tensor.matmul(
                                out=psY[:],
                                lhsT=xt_t[:, bi * 512 + j * P : bi * 512 + (j + 1) * P],
                                rhs=w_k[:, j * D : (j + 1) * D],
                                start=(j == 0),
                                stop=(j == 3),
                            )
                        nc.vector.tensor_add(
                            out=ys[:, (bt - done) * D : (bt - done + 1) * D],
                            in0=psY[:],
                            in1=b_k[:],
                        )
                        if bt + 1 in bset:
                            nc.scalar.dma_start(
                                out=y_d[:, (off[k] + done) * 512 : (off[k] + bt + 1) * 512],
                                in_=ys[:, : (bt + 1 - done) * D],
                            )
                            done = bt + 1
                            ys = None
                if k + SKEW < EL:
                    pend.append(load_x(k + SKEW))
                if k + WSKEW < EL:
                    wpend.append(load_w(k + WSKEW))
    nc.compile()
    return nc


def _ensure_ntff_hook():
    """The agent image's antenv lacks axon_hooks; shim it and install the
    ctypes NTFF profiling hook so trace=True works under axon."""
    import types

    try:
        from antenv import axon_hooks  # noqa: F401
        return
    except ImportError:
        pass
    mod = types.ModuleType("antenv.axon_hooks")
    _h = {"hook": None}
    mod.set_axon_ntff_profile_hook = lambda h: _h.update(hook=h)
    mod.get_axon_ntff_profile_hook = lambda: _h["hook"]
    sys.modules["antenv.axon_hooks"] = mod
    import antenv

    antenv.axon_hooks = mod
    try:
        if "/root/.axon_site" not in sys.path:
            sys.path.insert(0, "/root/.axon_site")
        from trn_agent_boot.trn_boot import _ntff_profile_via_ctypes

        hook = _ntff_profile_via_ctypes("/opt/axon/libaxon_pjrt.so")
        if hook is not None:
            mod.set_axon_ntff_profile_hook(hook)
    except Exception:
        pass


def kernel(x, weight, bias, expert_ids):
    global last_result
    from concourse import bass_utils
    from concourse.bass_utils import run_bass_kernel_spmd

    x = np.asarray(x, dtype=np.float32)
    weight = np.asarray(weight, dtype=np.float32)
    bias = np.asarray(bias, dtype=np.float32)
    expert_ids = np.asarray(expert_ids, dtype=np.int32)

    # ---- host routing: tokens sorted by expert, position within expert ----
    order = np.argsort(expert_ids, kind="stable")
    ids_s = expert_ids[order]
    counts = np.bincount(expert_ids, minlength=E)
    starts = np.cumsum(counts) - counts
    pos_s = np.arange(T, dtype=np.int64) - starts[ids_s]
    sel = pos_s < CAPD  # tokens the device computes

    # sort experts by count desc; rank r -> (slot r//NC, core r%NC)
    counts_c = np.minimum(counts, CAPD)
    rank = np.argsort(-counts_c, kind="stable")
    perm = rank.reshape(EL, NC)  # perm[slot, core] = expert id
    nt_slot = tuple(
        max(1, int(-(-counts_c[perm[k]].max() // 128))) for k in range(EL)
    )
    off = [0]
    for nt in nt_slot:
        off.append(off[-1] + nt)
    ntot = off[-1]

    if nt_slot not in _cache:
        _cache[nt_slot] = _build_program(nt_slot)
    nc = _cache[nt_slot]

    # ---- pack x: [E, CAPD, D] fp16, then to [E, 128 i_lo, bt, j, 128 t_lo] ----
    x16 = x.astype(np.float16)
    buf = np.zeros((E, CAPD, D), np.float16)
    buf[ids_s[sel], pos_s[sel]] = x16[order[sel]]
    xt = np.ascontiguousarray(
        buf.reshape(E, NTMAX, P, 4, P).transpose(0, 4, 1, 3, 2)
    ).reshape(E, P, NTMAX * 512)

    # ---- weights: [E, O, I] -> W^T tile layout [E, 128 i_lo, 4 j * 512 o] ----
    wt16 = np.ascontiguousarray(weight.transpose(0, 2, 1)).astype(np.float16)
    wt16 = np.ascontiguousarray(
        wt16.reshape(E, 4, P, D).transpose(0, 2, 1, 3)
    ).reshape(E, P, 4 * D)

    in_maps = []
    for c in range(NC):
        ex = perm[:, c]
        in_maps.append(
            {
                "x": np.concatenate(
                    [xt[ex[k]][:, : nt_slot[k] * 512] for k in range(EL)], axis=1
                ),
                "wt": np.ascontiguousarray(
                    wt16[ex].transpose(1, 0, 2).reshape(P, EL * 4 * D)
                ),
                "bias": np.ascontiguousarray(
                    bias[ex].reshape(1, EL * D).astype(np.float16)
                ),
            }
        )

    trace = bool(int(os.environ.get("KERNEL_TRACE", "0")))
    kwargs = {}
    if trace:
        _ensure_ntff_hook()
        bass_utils.upload_artifacts = lambda tmpdir: "local://" + tmpdir
        tdir = os.environ.get("KERNEL_TRACE_DIR")
        if tdir:
            os.makedirs(tdir, exist_ok=True)
            kwargs["tmpdir"] = tdir
    res = run_bass_kernel_spmd(
        nc, in_maps, core_ids=list(range(NC)), trace=trace, **kwargs
    )
    last_result = res

    # ---- unpack: y blocks [128 t_lo, bt*512+o] per (slot, core) -> [E, CAPD, D]
    ypad = np.zeros((E, P, NTMAX * 512), np.float16)
    for c in range(NC):
        yc = res.results[c]["y"]
        for k in range(EL):
            ypad[perm[k, c]][:, : nt_slot[k] * 512] = yc[
                :, off[k] * 512 : (off[k] + nt_slot[k]) * 512
            ]
    yall = (
        ypad.reshape(E, P, NTMAX, D).transpose(0, 2, 1, 3).reshape(E, CAPD, D)
    )
    out = np.zeros((T, D), np.float32)
    out[order[sel]] = yall[ids_s[sel], pos_s[sel]].astype(np.float32)

    # tokens beyond device capacity but within global capacity: exact host math
    ovf = (~sel) & (pos_s < CAPG)
    for t_idx in order[ovf]:
        e = expert_ids[t_idx]
        out[t_idx] = weight[e] @ x[t_idx] + bias[e]
    return out
